# revision 7
# baseline (speedup 1.0000x reference)
"""Trainium2 Bass kernel for nn_AdaptiveAutoCorrelation (8-core data-parallel).

Per-core (one batch element b):
  1. LayerNorm(q), LayerNorm(k) over E=64 segments; layout [l partitions, (h,e) free]
  2. rFFT of q,k at 3 scales as matmuls against host-built DFT matrices
     (avg-pool folded into the DFT matrix rows; sigmoid(freq_filter),
     softmax(scale_weights), linear interp and the (H,E)-mean all folded into
     the inverse-DFT matrix). float32r (TF32-rate) matmuls.
  3. Spectral whitening qf*conj(kf)/|kf| and reduction over (h,e) rows via
     tensor_tensor_reduce -> S[f] per scale.
  4. mean_corr[1,L] = S @ M (tiny matmul, accumulated in PSUM).
  5. top-7 via DVE max/max_index, softmax, then 7 dynamically-offset DMA
     "rolls" of a doubled values buffer + weighted MAC.

The (H,E)-mean/clip swap is exact for this model: |corr| <= ~3.7 << 10.
"""
import math

import numpy as np

L = 1536
H, E = 8, 64
R = H * E  # 512
B = 8
NT = L // 128  # 12 l-tiles
SCALES = [1, 2, 4]
FBINS = [L // s // 2 + 1 for s in SCALES]  # [769, 385, 193]
FT = [(f + 127) // 128 for f in FBINS]  # f-tiles per re/im block: [7, 4, 2]
NFT = 2 * sum(FT)  # 26 total f-tiles
TOPK = int(math.log(L))  # 7
LN_EPS = 1e-5

# global ftile index bases: per scale, re tiles then im tiles
_FT_BASE = []
_acc = 0
for _s in range(len(SCALES)):
    _FT_BASE.append((_acc, _acc + FT[_s]))
    _acc += 2 * FT[_s]

_CACHE = {}


def _build_constants(scale_weights, frequency_filter):
    """D tiled [NFT, NT, 128, 128] and M tiled [NFT, 3, 128, 512], float32."""
    f_sig = 1.0 / (1.0 + np.exp(-np.float64(frequency_filter[0])))
    sw = np.asarray(scale_weights[: len(SCALES)], np.float64)
    w = np.exp(sw - sw.max())
    w = w / w.sum()

    D = np.zeros((L, NFT * 128), np.float64)
    M = np.zeros((NFT * 128, L), np.float64)
    for si, s in enumerate(SCALES):
        Ls = L // s
        F = FBINS[si]
        reb, imb = _FT_BASE[si]
        t = np.arange(L)[:, None] // s
        f = np.arange(F)[None, :]
        ang = 2.0 * np.pi * t * f / Ls
        D[:, reb * 128 : reb * 128 + F] = np.cos(ang) / s
        D[:, imb * 128 : imb * 128 + F] = -np.sin(ang) / s

        # inverse rfft matrix rows, then linear-interp to L, fold w*f_sig/R
        tt = np.arange(Ls)[None, :]
        cf = np.where((f.T == 0) | (f.T == F - 1), 1.0, 2.0)
        ang2 = 2.0 * np.pi * f.T * tt / Ls
        Mre = cf * np.cos(ang2) / Ls  # [F, Ls]
        Mim = -cf * np.sin(ang2) / Ls
        if Ls != L:
            P = np.zeros((Ls, L))
            co = np.clip((np.arange(L) + 0.5) * (Ls / L) - 0.5, 0, Ls - 1)
            lo = np.floor(co).astype(int)
            hi = np.minimum(lo + 1, Ls - 1)
            fr = co - lo
            P[lo, np.arange(L)] += 1 - fr
            P[hi, np.arange(L)] += fr
            Mre = Mre @ P
            Mim = Mim @ P
        scale = w[si] * f_sig / R
        M[reb * 128 : reb * 128 + F] = Mre * scale
        M[imb * 128 : imb * 128 + F] = Mim * scale

    D_t = (
        D.reshape(NT, 128, NFT, 128).transpose(2, 0, 1, 3).astype(np.float32).copy()
    )
    M_t = (
        M.reshape(NFT, 128, 3, 512).transpose(0, 2, 1, 3).astype(np.float32).copy()
    )
    return D_t, M_t


def _build_graph():
    import concourse.bacc as bacc
    import concourse.bass as bass
    import concourse.mybir as mybir
    import concourse.tile as tile

    AF = mybir.ActivationFunctionType
    OP = mybir.AluOpType
    f32 = mybir.dt.float32
    f32r = mybir.dt.float32r

    nc = bacc.Bacc("TRN2", debug=False)
    q_d = nc.dram_tensor("q", [NT, 128, R], f32, kind="ExternalInput")
    k_d = nc.dram_tensor("k", [NT, 128, R], f32, kind="ExternalInput")
    vv_d = nc.dram_tensor("vv", [2 * L, R], f32, kind="ExternalInput")
    d_d = nc.dram_tensor("dmat", [NFT, NT, 128, 128], f32r, kind="ExternalInput")
    m_d = nc.dram_tensor("mmat", [NFT, 3, 128, 512], f32r, kind="ExternalInput")
    o_d = nc.dram_tensor("out", [NT, 128, R], f32, kind="ExternalOutput")

    def r32(ap):
        return ap.bitcast(f32r)

    with tile.TileContext(nc) as tc:
        with (
            tc.tile_pool(name="qk", bufs=1) as qkpool,
            tc.tile_pool(name="work", bufs=3) as wpool,
            tc.tile_pool(name="small", bufs=1) as spool,
        ):
            # ---- bias constants ----
            eps_ln = spool.tile([128, 1], f32, tag="eps_ln")
            nc.vector.memset(eps_ln[:], LN_EPS)
            eps_mag = spool.tile([128, 1], f32, tag="eps_mag")
            nc.vector.memset(eps_mag[:], 1e-30)

            # ---- load + layernorm ----
            qn = []
            kn = []
            for name, src, dst in (("q", q_d, qn), ("k", k_d, kn)):
                for kt in range(NT):
                    raw = wpool.tile([128, R], f32, tag="raw")
                    nc.sync.dma_start(raw[:], src.ap()[kt])
                    xs = raw[:].rearrange("p (h e) -> p h e", e=E)
                    stat = wpool.tile([128, 8], f32, tag="stat")
                    sq = wpool.tile([128, R], f32, tag="sq")
                    stat2 = wpool.tile([128, 8], f32, tag="stat2")
                    nc.vector.tensor_reduce(stat[:], xs, mybir.AxisListType.X, OP.add)
                    nc.scalar.activation(sq[:], raw[:], AF.Square)
                    nc.vector.tensor_reduce(
                        stat2[:], sq[:].rearrange("p (h e) -> p h e", e=E),
                        mybir.AxisListType.X, OP.add,
                    )
                    mean = wpool.tile([128, 8], f32, tag="mean")
                    nc.vector.tensor_scalar_mul(mean[:], stat[:], 1.0 / E)
                    m2 = wpool.tile([128, 8], f32, tag="m2")
                    nc.vector.tensor_mul(m2[:], mean[:], mean[:])
                    var = wpool.tile([128, 8], f32, tag="var")
                    nc.vector.scalar_tensor_tensor(
                        var[:], stat2[:], 1.0 / E, m2[:], op0=OP.mult, op1=OP.subtract
                    )
                    std = wpool.tile([128, 8], f32, tag="std")
                    nc.scalar.activation(std[:], var[:], AF.Sqrt, bias=eps_ln[:])
                    rstd = wpool.tile([128, 8], f32, tag="rstd")
                    nc.vector.reciprocal(rstd[:], std[:])
                    xn = qkpool.tile([128, R], f32, tag=f"{name}n{kt}")
                    mean3 = mean[:].rearrange("p (h o) -> p h o", o=1)
                    rstd3 = rstd[:].rearrange("p (h o) -> p h o", o=1)
                    xs_b, mean_b = bass.broadcast_tensor_aps(xs, mean3)
                    nc.vector.tensor_tensor(
                        r32(xn[:].rearrange("p (h e) -> p h e", e=E)),
                        xs_b, mean_b, OP.subtract,
                    )
                    xn3 = xn[:].rearrange("p (h e) -> p h e", e=E)
                    xn_b, rstd_b = bass.broadcast_tensor_aps(xn3, rstd3)
                    nc.vector.tensor_tensor(r32(xn_b), xn_b, rstd_b, OP.mult)
                    dst.append(xn)

            # ---- DFT + spectral + streaming irfft accumulate ----
            S_big = spool.tile([128, 32], f32, tag="sbig")
            S_r32 = spool.tile([128, 32], f32r, tag="sr32")
            with (
                tc.tile_pool(name="psum", bufs=1, space="PSUM") as pp,
                tc.tile_pool(name="mcpsum", bufs=1, space="PSUM") as mcp,
                tc.tile_pool(name="dstream", bufs=4) as dpool,
                tc.tile_pool(name="spec", bufs=2) as scp,
            ):
                mc_ps = [
                    mcp.tile([1, 512], f32, tag=f"mc{nt}", name=f"mc{nt}")
                    for nt in range(3)
                ]
                n_pairs = sum(FT)
                pair_list = []
                for si in range(len(SCALES)):
                    reb, imb = _FT_BASE[si]
                    for j in range(FT[si]):
                        pair_list.append((reb + j, imb + j))
                first_mm = True
                for pi, (ftr, fti) in enumerate(pair_list):
                    qre = pp.tile([128, 512], f32, tag="qre")
                    qim = pp.tile([128, 512], f32, tag="qim")
                    kre = pp.tile([128, 512], f32, tag="kre")
                    kim = pp.tile([128, 512], f32, tag="kim")
                    for kt in range(NT):
                        dre = dpool.tile([128, 128], f32r, tag="dre")
                        dim = dpool.tile([128, 128], f32r, tag="dim")
                        nc.sync.dma_start(dre[:], d_d.ap()[ftr, kt])
                        nc.sync.dma_start(dim[:], d_d.ap()[fti, kt])
                        st = kt == 0
                        sp = kt == NT - 1
                        nc.tensor.matmul(qre[:], dre[:], r32(qn[kt][:]), start=st, stop=sp)
                        nc.tensor.matmul(kre[:], dre[:], r32(kn[kt][:]), start=st, stop=sp)
                        nc.tensor.matmul(qim[:], dim[:], r32(qn[kt][:]), start=st, stop=sp)
                        nc.tensor.matmul(kim[:], dim[:], r32(kn[kt][:]), start=st, stop=sp)
                    # spectral: S += sum_r qf * conj(kf) / (|kf|)
                    t1 = scp.tile([128, 512], f32, tag="t1")
                    t2 = scp.tile([128, 512], f32, tag="t2")
                    nc.scalar.activation(t1[:], kre[:], AF.Square)
                    nc.scalar.activation(t2[:], kim[:], AF.Square)
                    nc.vector.tensor_add(t1[:], t1[:], t2[:])
                    nc.scalar.activation(t1[:], t1[:], AF.Sqrt, bias=eps_mag[:, 0:1])
                    rs = scp.tile([128, 512], f32, tag="rs")
                    nc.vector.reciprocal(rs[:], t1[:])
                    khr = scp.tile([128, 512], f32, tag="khr")
                    khi = scp.tile([128, 512], f32, tag="khi")
                    nc.vector.tensor_mul(khr[:], kre[:], rs[:])
                    nc.vector.tensor_mul(khi[:], kim[:], rs[:])
                    scr = scp.tile([128, 512], f32, tag="scr")
                    scr2 = scp.tile([128, 512], f32, tag="scr2")
                    a1 = scp.tile([128, 1], f32, tag="a1")
                    a2 = scp.tile([128, 1], f32, tag="a2")
                    a3 = scp.tile([128, 1], f32, tag="a3")
                    a4 = scp.tile([128, 1], f32, tag="a4")
                    nc.vector.scalar_tensor_tensor(
                        scr[:], qre[:], 0.0, khr[:], op0=OP.bypass, op1=OP.mult,
                        accum_out=a1[:],
                    )
                    nc.vector.scalar_tensor_tensor(
                        scr2[:], qim[:], 0.0, khi[:], op0=OP.bypass, op1=OP.mult,
                        accum_out=a2[:],
                    )
                    nc.vector.tensor_add(S_big[:, ftr : ftr + 1], a1[:], a2[:])
                    nc.vector.scalar_tensor_tensor(
                        scr[:], qim[:], 0.0, khr[:], op0=OP.bypass, op1=OP.mult,
                        accum_out=a3[:],
                    )
                    nc.vector.scalar_tensor_tensor(
                        scr2[:], qre[:], 0.0, khi[:], op0=OP.bypass, op1=OP.mult,
                        accum_out=a4[:],
                    )
                    nc.vector.tensor_sub(S_big[:, fti : fti + 1], a3[:], a4[:])
                    nc.vector.tensor_copy(S_r32[:, ftr : ftr + 1], S_big[:, ftr : ftr + 1])
                    nc.vector.tensor_copy(S_r32[:, fti : fti + 1], S_big[:, fti : fti + 1])
                    # streaming irfft: mean_corr accumulation over this pair
                    for ft in (ftr, fti):
                        for nt in range(3):
                            mtile = dpool.tile([128, 512], f32r, tag="mtile")
                            nc.sync.dma_start(mtile[:], m_d.ap()[ft, nt])
                            nc.tensor.matmul(
                                mc_ps[nt][:],
                                S_r32[:, ft : ft + 1],
                                mtile[:],
                                start=first_mm,
                                stop=(pi == n_pairs - 1 and ft == fti and nt == 2),
                                skip_group_check=True,
                            )
                        first_mm = False if ft == ftr else first_mm
                    first_mm = False

                # ---- mean_corr -> top-7 -> softmax ----
                mc_row = spool.tile([1, L], f32, tag="mcrow")
                for nt in range(3):
                    nc.scalar.activation(
                        mc_row[:, nt * 512 : (nt + 1) * 512], mc_ps[nt][:], AF.Copy
                    )

            mc8 = spool.tile([1, 8], f32, tag="mc8")
            mcidx = spool.tile([1, 8], mybir.dt.uint32, tag="mcidx")
            nc.vector.max(mc8[:], mc_row[:])
            nc.vector.max_index(mcidx[:], mc8[:], mc_row[:])
            negmax = spool.tile([1, 1], f32, tag="negmax")
            nc.vector.tensor_scalar_mul(negmax[:], mc8[:, 0:1], -1.0)
            e7 = spool.tile([1, TOPK], f32, tag="e7")
            nc.scalar.activation(e7[:], mc8[:, 0:TOPK], AF.Exp, bias=negmax[:])
            ssum = spool.tile([1, 1], f32, tag="ssum")
            nc.vector.tensor_reduce(ssum[:], e7[:], mybir.AxisListType.X, OP.add)
            rsum = spool.tile([1, 1], f32, tag="rsum")
            nc.vector.reciprocal(rsum[:], ssum[:])
            nw = spool.tile([1, TOPK], f32, tag="nw")
            nc.vector.tensor_scalar_mul(nw[:], e7[:], rsum[:, 0:1])
            nw128 = spool.tile([128, TOPK], f32, tag="nw128")
            nc.gpsimd.partition_broadcast(nw128[:], nw[:])

            # ---- gather (indirect row gathers) + weighted MAC ----
            acc = spool.tile([128, NT, 512], f32, tag="acc")
            iotas = []
            for lt in range(NT):
                it = spool.tile([128, 1], mybir.dt.uint32, tag=f"iota{lt}",
                                name=f"iota{lt}")
                nc.gpsimd.iota(it[:], pattern=[[0, 1]], base=128 * lt,
                               channel_multiplier=1)
                iotas.append(it)
            with tc.tile_pool(name="gather", bufs=4) as gpool:
                for kk in range(TOPK):
                    d128 = gpool.tile([128, 1], mybir.dt.uint32, tag="d128")
                    nc.gpsimd.partition_broadcast(d128[:], mcidx[:, kk : kk + 1])
                    for lt in range(NT):
                        idx = gpool.tile([128, 1], mybir.dt.uint32, tag="idx")
                        nc.vector.tensor_tensor(idx[:], iotas[lt][:], d128[:], OP.add)
                        slot = gpool.tile([128, 512], f32, tag="slot")
                        nc.gpsimd.indirect_dma_start(
                            out=slot[:],
                            out_offset=None,
                            in_=vv_d.ap(),
                            in_offset=bass.IndirectOffsetOnAxis(ap=idx[:, 0:1], axis=0),
                        )
                        if kk == 0:
                            nc.vector.tensor_scalar_mul(
                                acc[:, lt, :], slot[:], nw128[:, 0:1]
                            )
                        else:
                            nc.vector.scalar_tensor_tensor(
                                acc[:, lt, :], slot[:], nw128[:, kk : kk + 1],
                                acc[:, lt, :], op0=OP.mult, op1=OP.add,
                            )
            for kt in range(NT):
                nc.sync.dma_start(o_d.ap()[kt], acc[:, kt, :])

    nc.compile()
    return nc


def _get_graph():
    if "nc" not in _CACHE:
        _CACHE["nc"] = _build_graph()
    return _CACHE["nc"]


def kernel(queries, keys, values, scale_weights, frequency_filter, attn_mask=None):
    from concourse.bass_utils import run_bass_kernel_spmd

    nc = _get_graph()
    D_t, M_t = _build_constants(
        np.asarray(scale_weights, np.float64), np.asarray(frequency_filter, np.float64)
    )
    q = np.ascontiguousarray(np.asarray(queries, np.float32).reshape(B, NT, 128, R))
    k = np.ascontiguousarray(np.asarray(keys, np.float32).reshape(B, NT, 128, R))
    v = np.asarray(values, np.float32).reshape(B, L, R)
    vv = np.concatenate([v, v], axis=1)  # [B, 2L, R]

    in_maps = []
    for b in range(B):
        in_maps.append(
            {
                "q": q[b],
                "k": k[b],
                "vv": np.ascontiguousarray(vv[b]),
                "dmat": D_t,
                "mmat": M_t,
            }
        )
    res = run_bass_kernel_spmd(nc, in_maps, core_ids=list(range(B)))
    out = np.stack([res.results[b]["out"].reshape(L, H, E) for b in range(B)])
    return out.astype(np.float32)


# revision 10
# speedup vs baseline: 1.0002x; 1.0002x over previous
"""Trainium2 Bass kernel for nn_AdaptiveAutoCorrelation (8-core data-parallel).

Per-core (one batch element b):
  1. LayerNorm(q), LayerNorm(k) over E=64 segments; layout [l partitions, (h,e) free].
     Stats via GpSimd pool_avg + ACT Square; apply on DVE.
  2. Avg-pool to scales 2,4 via PE matmuls (P2a/P2b packing matrices).
  3. rFFT of q,k per scale as float32r matmuls against host-built DFT matrices.
  4. Spectral whitening qf*conj(kf)/|kf| + (h,e)-reduction via stt accum -> S[f].
  5. mean_corr = S @ M (irfft+interp+scale-weights+mean folded into M), streamed
     in PSUM alongside the DFT pairs.
  6. top-7 via DVE max/max_index, softmax, indirect row gathers of doubled
     values (bf16) + weighted MAC.

The (H,E)-mean/clip swap is exact for this model: |corr| <= ~3.7 << 10.
"""
import math

import numpy as np

L = 1536
H, E = 8, 64
R = H * E  # 512
B = 8
NT = L // 128  # 12 l-tiles
SCALES = [1, 2, 4]
KT = [12, 6, 3]  # contraction tiles per scale (pooled-first)
FBINS = [L // s // 2 + 1 for s in SCALES]  # [769, 385, 193]
FT = [(f + 127) // 128 for f in FBINS]  # f-tiles per re/im block: [7, 4, 2]
NFT = 2 * sum(FT)  # 26 total f-tiles
TOPK = int(math.log(L))  # 7
LN_EPS = 1e-5

# global ftile index bases (for S / M layout): per scale, re tiles then im tiles
_FT_BASE = []
_acc = 0
for _s in range(len(SCALES)):
    _FT_BASE.append((_acc, _acc + FT[_s]))
    _acc += 2 * FT[_s]

# flat D-tile index: for si, local_ft in [0, 2*FT[si]), kt in [0, KT[si])
_D_IDX = {}
_n = 0
for _si in range(len(SCALES)):
    for _lf in range(2 * FT[_si]):
        for _kt in range(KT[_si]):
            _D_IDX[(_si, _lf, _kt)] = _n
            _n += 1
ND_TILES = _n  # 228

_CACHE = {}


def _build_constants(scale_weights, frequency_filter):
    """D tiles [ND_TILES,128,128], M tiles [NFT,3,128,512], pool mats [2,128,128]."""
    f_sig = 1.0 / (1.0 + np.exp(-np.float64(frequency_filter[0])))
    sw = np.asarray(scale_weights[: len(SCALES)], np.float64)
    w = np.exp(sw - sw.max())
    w = w / w.sum()

    D_t = np.zeros((ND_TILES, 128, 128), np.float32)
    M = np.zeros((NFT * 128, L), np.float64)
    for si, s in enumerate(SCALES):
        Ls = L // s
        F = FBINS[si]
        nf = FT[si]
        t = np.arange(Ls)[:, None]
        f = np.arange(F)[None, :]
        ang = 2.0 * np.pi * t * f / Ls
        Dre = np.zeros((Ls, nf * 128))
        Dim = np.zeros((Ls, nf * 128))
        Dre[:, :F] = np.cos(ang)
        Dim[:, :F] = -np.sin(ang)
        for lf in range(2 * nf):
            blk = Dre if lf < nf else Dim
            j = lf % nf
            for kt in range(KT[si]):
                D_t[_D_IDX[(si, lf, kt)]] = blk[
                    kt * 128 : (kt + 1) * 128, j * 128 : (j + 1) * 128
                ].astype(np.float32)

        reb, imb = _FT_BASE[si]
        tt = np.arange(Ls)[None, :]
        cf = np.where((f.T == 0) | (f.T == F - 1), 1.0, 2.0)
        ang2 = 2.0 * np.pi * f.T * tt / Ls
        Mre = cf * np.cos(ang2) / Ls  # [F, Ls]
        Mim = -cf * np.sin(ang2) / Ls
        if Ls != L:
            P = np.zeros((Ls, L))
            co = np.clip((np.arange(L) + 0.5) * (Ls / L) - 0.5, 0, Ls - 1)
            lo = np.floor(co).astype(int)
            hi = np.minimum(lo + 1, Ls - 1)
            fr = co - lo
            P[lo, np.arange(L)] += 1 - fr
            P[hi, np.arange(L)] += fr
            Mre = Mre @ P
            Mim = Mim @ P
        scale = w[si] * f_sig / R
        M[reb * 128 : reb * 128 + F] = Mre * scale
        M[imb * 128 : imb * 128 + F] = Mim * scale

    M_t = (
        M.reshape(NFT, 128, 3, 512).transpose(0, 2, 1, 3).astype(np.float32).copy()
    )
    # pool-by-2 packing matrices: P2a -> out cols [0,64), P2b -> [64,128)
    P2 = np.zeros((2, 128, 128), np.float32)
    for t_ in range(128):
        P2[0, t_, t_ // 2] = 0.5
        P2[1, t_, 64 + t_ // 2] = 0.5
    return D_t, M_t, P2


def _build_graph():
    import concourse.bacc as bacc
    import concourse.bass as bass
    import concourse.mybir as mybir
    import concourse.tile as tile

    AF = mybir.ActivationFunctionType
    OP = mybir.AluOpType
    f32 = mybir.dt.float32
    f32r = mybir.dt.float32r
    bf16 = mybir.dt.bfloat16

    nc = bacc.Bacc("TRN2", debug=False)
    q_d = nc.dram_tensor("q", [NT, 128, R], f32, kind="ExternalInput")
    k_d = nc.dram_tensor("k", [NT, 128, R], f32, kind="ExternalInput")
    vv_d = nc.dram_tensor("vv", [2 * L, R], bf16, kind="ExternalInput")
    d_d = nc.dram_tensor("dmat", [ND_TILES, 128, 128], f32r, kind="ExternalInput")
    m_d = nc.dram_tensor("mmat", [NFT, 3, 128, 512], f32r, kind="ExternalInput")
    p_d = nc.dram_tensor("pmat", [2, 128, 128], f32r, kind="ExternalInput")
    o_d = nc.dram_tensor("out", [NT, 128, R], f32, kind="ExternalOutput")

    def r32(ap):
        return ap.bitcast(f32r)

    with tile.TileContext(nc) as tc:
        with (
            tc.tile_pool(name="qk", bufs=1) as qkpool,
            tc.tile_pool(name="work", bufs=3) as wpool,
            tc.tile_pool(name="small", bufs=1) as spool,
        ):
            eps_ln = spool.tile([128, 1], f32, tag="eps_ln")
            nc.vector.memset(eps_ln[:], LN_EPS)
            eps_mag = spool.tile([128, 1], f32, tag="eps_mag")
            nc.vector.memset(eps_mag[:], 1e-30)
            p2a = spool.tile([128, 128], f32r, tag="p2a")
            p2b = spool.tile([128, 128], f32r, tag="p2b")
            nc.sync.dma_start(p2a[:], p_d.ap()[0])
            nc.sync.dma_start(p2b[:], p_d.ap()[1])

            # ---- load + layernorm ----
            xn = {}  # (name, si, kt) -> tile
            for name, src in (("q", q_d), ("k", k_d)):
                for kt in range(NT):
                    raw = wpool.tile([128, R], f32, tag="raw")
                    nc.sync.dma_start(raw[:], src.ap()[kt])
                    xs = raw[:].rearrange("p (h e) -> p h e", e=E)
                    sq = wpool.tile([128, R], f32, tag="sq")
                    nc.scalar.activation(sq[:], raw[:], AF.Square)
                    stat = wpool.tile([128, 8], f32, tag="stat")
                    nc.vector.tensor_reduce(stat[:], xs, mybir.AxisListType.X, OP.add)
                    msq = wpool.tile([128, 8], f32, tag="msq")
                    nc.vector.tensor_reduce(
                        msq[:], sq[:].rearrange("p (h e) -> p h e", e=E),
                        mybir.AxisListType.X, OP.add,
                    )
                    mean = wpool.tile([128, 8], f32, tag="mean")
                    nc.vector.tensor_scalar_mul(mean[:], stat[:], 1.0 / E)
                    m2 = wpool.tile([128, 8], f32, tag="m2")
                    nc.vector.tensor_mul(m2[:], mean[:], mean[:])
                    var = wpool.tile([128, 8], f32, tag="var")
                    nc.vector.scalar_tensor_tensor(
                        var[:], msq[:], 1.0 / E, m2[:], op0=OP.mult, op1=OP.subtract
                    )
                    std = wpool.tile([128, 8], f32, tag="std")
                    nc.scalar.activation(std[:], var[:], AF.Sqrt, bias=eps_ln[:])
                    rstd = wpool.tile([128, 8], f32, tag="rstd")
                    nc.vector.reciprocal(rstd[:], std[:])
                    t_xn = qkpool.tile(
                        [128, R], f32, tag=f"{name}n{kt}", name=f"{name}n{kt}"
                    )
                    mean3 = mean[:].rearrange("p (h o) -> p h o", o=1)
                    rstd3 = rstd[:].rearrange("p (h o) -> p h o", o=1)
                    xs_b, mean_b = bass.broadcast_tensor_aps(xs, mean3)
                    nc.vector.tensor_tensor(
                        r32(t_xn[:].rearrange("p (h e) -> p h e", e=E)),
                        xs_b, mean_b, OP.subtract,
                    )
                    xn3 = t_xn[:].rearrange("p (h e) -> p h e", e=E)
                    xn_b, rstd_b = bass.broadcast_tensor_aps(xn3, rstd3)
                    nc.vector.tensor_tensor(r32(xn_b), xn_b, rstd_b, OP.mult)
                    xn[(name, 0, kt)] = t_xn

            # ---- avg-pool to scales 2 and 4 via PE ----
            with tc.tile_pool(name="poolps", bufs=2, space="PSUM") as ppool:
                for name in ("q", "k"):
                    for si, nkt in ((1, 6), (2, 3)):
                        for j in range(nkt):
                            ps = ppool.tile([128, R], f32, tag="ps", name="ps")
                            s0 = xn[(name, si - 1, 2 * j)]
                            s1 = xn[(name, si - 1, 2 * j + 1)]
                            nc.tensor.matmul(
                                ps[:], p2a[:], r32(s0[:]), start=True, stop=False
                            )
                            nc.tensor.matmul(
                                ps[:], p2b[:], r32(s1[:]), start=False, stop=True
                            )
                            t2 = qkpool.tile(
                                [128, R], f32r, tag=f"{name}p{si}_{j}",
                                name=f"{name}p{si}_{j}",
                            )
                            nc.scalar.activation(t2[:], ps[:], AF.Copy)
                            xn[(name, si, j)] = t2

            # ---- DFT + spectral + streaming irfft (mean_corr) ----
            S_big = spool.tile([128, 32], f32, tag="sbig")
            S_r32 = spool.tile([128, 32], f32r, tag="sr32")
            with (
                tc.tile_pool(name="psum", bufs=1, space="PSUM") as pp,
                tc.tile_pool(name="mcpsum", bufs=1, space="PSUM") as mcp,
                tc.tile_pool(name="dstream", bufs=6) as dpool,
                tc.tile_pool(name="spec", bufs=2) as scp,
            ):
                mc_ps = [
                    mcp.tile([1, 512], f32, tag=f"mc{nt}", name=f"mc{nt}")
                    for nt in range(3)
                ]
                pair_list = []
                for si in range(len(SCALES)):
                    reb, imb = _FT_BASE[si]
                    for j in range(FT[si]):
                        pair_list.append((si, j, reb + j, imb + j))
                n_pairs = len(pair_list)
                first_mm = True
                for pi, (si, j, ftr, fti) in enumerate(pair_list):
                    nkt = KT[si]
                    qre = pp.tile([128, 512], f32, tag="qre", name="qre")
                    qim = pp.tile([128, 512], f32, tag="qim", name="qim")
                    kre = pp.tile([128, 512], f32, tag="kre", name="kre")
                    kim = pp.tile([128, 512], f32, tag="kim", name="kim")
                    for kt in range(nkt):
                        dre = dpool.tile([128, 128], f32r, tag="dre")
                        dim = dpool.tile([128, 128], f32r, tag="dim")
                        nc.sync.dma_start(dre[:], d_d.ap()[_D_IDX[(si, j, kt)]])
                        nc.sync.dma_start(
                            dim[:], d_d.ap()[_D_IDX[(si, FT[si] + j, kt)]]
                        )
                        st = kt == 0
                        sp = kt == nkt - 1
                        qx = xn[("q", si, kt)]
                        kx = xn[("k", si, kt)]
                        nc.tensor.matmul(qre[:], dre[:], r32(qx[:]), start=st, stop=sp)
                        nc.tensor.matmul(kre[:], dre[:], r32(kx[:]), start=st, stop=sp)
                        nc.tensor.matmul(qim[:], dim[:], r32(qx[:]), start=st, stop=sp)
                        nc.tensor.matmul(kim[:], dim[:], r32(kx[:]), start=st, stop=sp)
                    # drain psum fast (ACT), then spectral on SBUF
                    qreS = scp.tile([128, 512], f32, tag="qreS")
                    qimS = scp.tile([128, 512], f32, tag="qimS")
                    kreS = scp.tile([128, 512], f32, tag="kreS")
                    kimS = scp.tile([128, 512], f32, tag="kimS")
                    nc.scalar.activation(kreS[:], kre[:], AF.Copy)
                    nc.scalar.activation(kimS[:], kim[:], AF.Copy)
                    nc.scalar.activation(qreS[:], qre[:], AF.Copy)
                    nc.scalar.activation(qimS[:], qim[:], AF.Copy)
                    t1 = scp.tile([128, 512], f32, tag="t1")
                    t2 = scp.tile([128, 512], f32, tag="t2")
                    nc.scalar.activation(t1[:], kreS[:], AF.Square)
                    nc.scalar.activation(t2[:], kimS[:], AF.Square)
                    nc.vector.tensor_add(t1[:], t1[:], t2[:])
                    nc.scalar.activation(t1[:], t1[:], AF.Sqrt, bias=eps_mag[:, 0:1])
                    rs = scp.tile([128, 512], f32, tag="rs")
                    nc.vector.reciprocal(rs[:], t1[:])
                    khr = scp.tile([128, 512], f32, tag="khr")
                    khi = scp.tile([128, 512], f32, tag="khi")
                    nc.vector.tensor_mul(khr[:], kreS[:], rs[:])
                    nc.vector.tensor_mul(khi[:], kimS[:], rs[:])
                    scr = scp.tile([128, 512], f32, tag="scr")
                    scr2 = scp.tile([128, 512], f32, tag="scr2")
                    a1 = scp.tile([128, 1], f32, tag="a1")
                    a2 = scp.tile([128, 1], f32, tag="a2")
                    a3 = scp.tile([128, 1], f32, tag="a3")
                    a4 = scp.tile([128, 1], f32, tag="a4")
                    nc.vector.scalar_tensor_tensor(
                        scr[:], qreS[:], 0.0, khr[:], op0=OP.bypass, op1=OP.mult,
                        accum_out=a1[:],
                    )
                    nc.vector.scalar_tensor_tensor(
                        scr2[:], qimS[:], 0.0, khi[:], op0=OP.bypass, op1=OP.mult,
                        accum_out=a2[:],
                    )
                    nc.vector.tensor_add(S_big[:, ftr : ftr + 1], a1[:], a2[:])
                    nc.vector.scalar_tensor_tensor(
                        scr[:], qimS[:], 0.0, khr[:], op0=OP.bypass, op1=OP.mult,
                        accum_out=a3[:],
                    )
                    nc.vector.scalar_tensor_tensor(
                        scr2[:], qreS[:], 0.0, khi[:], op0=OP.bypass, op1=OP.mult,
                        accum_out=a4[:],
                    )
                    nc.vector.tensor_sub(S_big[:, fti : fti + 1], a3[:], a4[:])
                    nc.vector.tensor_copy(
                        S_r32[:, ftr : ftr + 1], S_big[:, ftr : ftr + 1]
                    )
                    nc.vector.tensor_copy(
                        S_r32[:, fti : fti + 1], S_big[:, fti : fti + 1]
                    )
                    # streaming irfft: accumulate mean_corr for this pair
                    for ft in (ftr, fti):
                        for nt in range(3):
                            mtile = dpool.tile([128, 512], f32r, tag="mtile")
                            nc.sync.dma_start(mtile[:], m_d.ap()[ft, nt])
                            nc.tensor.matmul(
                                mc_ps[nt][:], S_r32[:, ft : ft + 1], mtile[:],
                                start=first_mm,
                                stop=(pi == n_pairs - 1 and ft == fti and nt == 2),
                                skip_group_check=True,
                            )
                        first_mm = False

                mc_row = spool.tile([1, L], f32, tag="mcrow")
                for nt in range(3):
                    nc.scalar.activation(
                        mc_row[:, nt * 512 : (nt + 1) * 512], mc_ps[nt][:], AF.Copy
                    )

            # ---- top-7 + softmax ----
            mc8 = spool.tile([1, 8], f32, tag="mc8")
            mcidx = spool.tile([1, 8], mybir.dt.uint32, tag="mcidx")
            nc.vector.max(mc8[:], mc_row[:])
            nc.vector.max_index(mcidx[:], mc8[:], mc_row[:])
            negmax = spool.tile([1, 1], f32, tag="negmax")
            nc.vector.tensor_scalar_mul(negmax[:], mc8[:, 0:1], -1.0)
            e7 = spool.tile([1, TOPK], f32, tag="e7")
            nc.scalar.activation(e7[:], mc8[:, 0:TOPK], AF.Exp, bias=negmax[:])
            ssum = spool.tile([1, 1], f32, tag="ssum")
            nc.vector.tensor_reduce(ssum[:], e7[:], mybir.AxisListType.X, OP.add)
            rsum = spool.tile([1, 1], f32, tag="rsum")
            nc.vector.reciprocal(rsum[:], ssum[:])
            nw = spool.tile([1, TOPK], f32, tag="nw")
            nc.vector.tensor_scalar_mul(nw[:], e7[:], rsum[:, 0:1])
            nw128 = spool.tile([128, TOPK], f32, tag="nw128")
            nc.gpsimd.partition_broadcast(nw128[:], nw[:])

            # ---- gather (indirect row gathers, bf16) + weighted MAC ----
            iotas = []
            for lt in range(NT):
                it = spool.tile(
                    [128, 1], mybir.dt.uint32, tag=f"iota{lt}", name=f"iota{lt}"
                )
                nc.gpsimd.iota(
                    it[:], pattern=[[0, 1]], base=128 * lt, channel_multiplier=1
                )
                iotas.append(it)
            with tc.tile_pool(name="gather", bufs=4) as gpool:
                acc = gpool.tile([128, NT, 512], f32, tag="acc", bufs=1)
                for kk in range(TOPK):
                    d128 = gpool.tile([128, 1], mybir.dt.uint32, tag="d128")
                    nc.gpsimd.partition_broadcast(d128[:], mcidx[:, kk : kk + 1])
                    for lt in range(NT):
                        idx = gpool.tile([128, 1], mybir.dt.uint32, tag="idx")
                        nc.vector.tensor_tensor(idx[:], iotas[lt][:], d128[:], OP.add)
                        slot = gpool.tile([128, 512], bf16, tag="slot")
                        nc.gpsimd.indirect_dma_start(
                            out=slot[:],
                            out_offset=None,
                            in_=vv_d.ap(),
                            in_offset=bass.IndirectOffsetOnAxis(ap=idx[:, 0:1], axis=0),
                        )
                        if kk == 0:
                            nc.vector.tensor_scalar_mul(
                                acc[:, lt, :], slot[:], nw128[:, 0:1]
                            )
                        else:
                            nc.vector.scalar_tensor_tensor(
                                acc[:, lt, :], slot[:], nw128[:, kk : kk + 1],
                                acc[:, lt, :], op0=OP.mult, op1=OP.add,
                            )
                for kt in range(NT):
                    nc.sync.dma_start(o_d.ap()[kt], acc[:, kt, :])

    nc.compile()
    return nc


def _get_graph():
    if "nc" not in _CACHE:
        _CACHE["nc"] = _build_graph()
    return _CACHE["nc"]


def _make_in_maps(queries, keys, values, scale_weights, frequency_filter):
    import ml_dtypes

    D_t, M_t, P2 = _build_constants(
        np.asarray(scale_weights, np.float64), np.asarray(frequency_filter, np.float64)
    )
    q = np.ascontiguousarray(np.asarray(queries, np.float32).reshape(B, NT, 128, R))
    k = np.ascontiguousarray(np.asarray(keys, np.float32).reshape(B, NT, 128, R))
    v = np.asarray(values, np.float32).reshape(B, L, R)
    vv = np.concatenate([v, v], axis=1).astype(ml_dtypes.bfloat16)
    in_maps = []
    for b in range(B):
        in_maps.append(
            {
                "q": q[b],
                "k": k[b],
                "vv": np.ascontiguousarray(vv[b]),
                "dmat": D_t,
                "mmat": M_t,
                "pmat": P2,
            }
        )
    return in_maps


def kernel(queries, keys, values, scale_weights, frequency_filter, attn_mask=None):
    from concourse.bass_utils import run_bass_kernel_spmd

    nc = _get_graph()
    in_maps = _make_in_maps(queries, keys, values, scale_weights, frequency_filter)
    res = run_bass_kernel_spmd(nc, in_maps, core_ids=list(range(B)))
    out = np.stack([res.results[b]["out"].reshape(L, H, E) for b in range(B)])
    return out.astype(np.float32)
